# revision 2
# baseline (speedup 1.0000x reference)
"""DMTet geometry extraction on 8 Trainium2 NeuronCores.

Pipeline:
  Launch A (device, verts sharded): pos = verts + tanh(deform)/grid_res,
      packed pos_sdf table [Nv,4] + occupancy (sdf>0) per vertex.
  Host: tet occupancy codes -> valid (crossing) tets -> crossing edges ->
      canonical-sort + dedup (the global `unique` step) -> interp edge list,
      idx_map / triangle-table rows, and the random-access row gathers
      (no fast large-table gather primitive exists in this Bass stack, so
      index-space work stays on host).
  Launch B (device, edges + tets sharded): linear interpolation along
      crossing edges (reciprocal/weights/lerp) and the 6-way triangle-table
      select producing face vertex ids.
"""
import os
import numpy as np

import concourse.bacc as bacc
import concourse.bass as bass
import concourse.mybir as mybir
import concourse.tile as tile
from concourse import bass_utils

NCORES = 8
F32 = mybir.dt.float32

TRIANGLE_TABLE = np.array([
    [-1, -1, -1, -1, -1, -1], [1, 0, 2, -1, -1, -1], [4, 0, 3, -1, -1, -1],
    [1, 4, 2, 1, 3, 4], [3, 1, 5, -1, -1, -1], [2, 3, 0, 2, 5, 3],
    [1, 4, 0, 1, 5, 4], [4, 2, 5, -1, -1, -1], [4, 5, 2, -1, -1, -1],
    [4, 1, 0, 4, 5, 1], [3, 2, 0, 3, 5, 2], [1, 3, 5, -1, -1, -1],
    [4, 1, 2, 4, 3, 1], [3, 0, 4, -1, -1, -1], [2, 0, 1, -1, -1, -1],
    [-1, -1, -1, -1, -1, -1]], dtype=np.int32)
NUM_TRI = np.array([0, 1, 1, 2, 1, 2, 2, 1, 1, 2, 2, 1, 2, 1, 1, 0],
                   dtype=np.int32)
EI = np.array([0, 0, 0, 1, 1, 2], dtype=np.int32)
EJ = np.array([1, 2, 3, 2, 3, 3], dtype=np.int32)
_BITS = ((np.arange(16)[:, None] >> np.arange(4)[None, :]) & 1).astype(np.int32)
CROSS = _BITS[:, EI] != _BITS[:, EJ]  # [16, 6] bool

EXEC_NS = []  # per-launch max-over-cores HW exec time (filled when tracing)


def _run(nc, in_maps):
    trace = os.environ.get("DMTET_TRACE") == "1"
    kw = {}
    if trace:
        kw = dict(trace=True, trace_cores=list(range(NCORES)))
    res = bass_utils.run_bass_kernel_spmd(
        nc, in_maps, core_ids=list(range(NCORES)), **kw)
    if trace:
        EXEC_NS.append(res.exec_time_ns)
    return res.results


def _build_launch_a(rows_pc, scale):
    """Per core: pos_sdf [rows_pc, 4] f32 and occ [rows_pc] f32 from
    verts/deform [rows_pc, 3] + sdf [rows_pc]. rows_pc % 128 == 0."""
    P = 128
    C = rows_pc // P
    nc = bacc.Bacc("TRN2", target_bir_lowering=False, debug=False)
    v = nc.dram_tensor("verts", [rows_pc, 3], F32, kind="ExternalInput").ap()
    d = nc.dram_tensor("deform", [rows_pc, 3], F32, kind="ExternalInput").ap()
    s = nc.dram_tensor("sdf", [rows_pc], F32, kind="ExternalInput").ap()
    ps = nc.dram_tensor("pos_sdf", [rows_pc, 4], F32, kind="ExternalOutput").ap()
    oc = nc.dram_tensor("occ", [rows_pc], F32, kind="ExternalOutput").ap()

    vv = v.rearrange("(p c) k -> p (c k)", p=P)
    dv = d.rearrange("(p c) k -> p (c k)", p=P)
    sv = s.rearrange("(p c) -> p c", p=P)
    pv = ps.rearrange("(p c) k -> p (c k)", p=P)
    ov = oc.rearrange("(p c) -> p c", p=P)

    with tile.TileContext(nc) as tc:
        with tc.tile_pool(name="sbuf", bufs=1) as pool:
            vt = pool.tile([P, C * 3], F32)
            dt = pool.tile([P, C * 3], F32)
            st = pool.tile([P, C], F32)
            nc.sync.dma_start(out=vt[:], in_=vv)
            nc.sync.dma_start(out=dt[:], in_=dv)
            nc.sync.dma_start(out=st[:], in_=sv)
            th = pool.tile([P, C * 3], F32)
            nc.scalar.activation(th[:], dt[:], mybir.ActivationFunctionType.Tanh)
            nc.vector.tensor_scalar_mul(th[:], th[:], float(scale))
            out4 = pool.tile([P, C * 4], F32)
            o3 = out4[:].rearrange("p (c k) -> p c k", k=4)
            v3 = vt[:].rearrange("p (c k) -> p c k", k=3)
            t3 = th[:].rearrange("p (c k) -> p c k", k=3)
            for k in range(3):
                nc.vector.tensor_tensor(
                    out=o3[:, :, k], in0=v3[:, :, k], in1=t3[:, :, k],
                    op=mybir.AluOpType.add)
            nc.vector.tensor_copy(out=o3[:, :, 3], in_=st[:])
            ot = pool.tile([P, C], F32)
            nc.vector.tensor_scalar(
                out=ot[:], in0=st[:], scalar1=0.0, scalar2=None,
                op0=mybir.AluOpType.is_gt)
            nc.sync.dma_start(out=pv, in_=out4[:])
            nc.sync.dma_start(out=ov, in_=ot[:])
    nc.compile()
    return nc


def _build_launch_b(mq, fq, ci, cf):
    """Per core: interp over X [mq, 8] -> VO [mq, 3]; face select over
    IM/TT [fq, 6] -> F6 [fq, 6]. mq % (128*ci) == 0, fq % (128*cf) == 0."""
    P = 128
    nti = mq // (P * ci)
    ntf = fq // (P * cf)
    nc = bacc.Bacc("TRN2", target_bir_lowering=False, debug=False)
    X = nc.dram_tensor("X", [mq, 8], F32, kind="ExternalInput").ap()
    IM = nc.dram_tensor("IM", [fq, 6], F32, kind="ExternalInput").ap()
    TT = nc.dram_tensor("TT", [fq, 6], F32, kind="ExternalInput").ap()
    VO = nc.dram_tensor("VO", [mq, 3], F32, kind="ExternalOutput").ap()
    F6 = nc.dram_tensor("F6", [fq, 6], F32, kind="ExternalOutput").ap()

    Xv = X.rearrange("(t p c) k -> t p (c k)", p=P, c=ci)
    VOv = VO.rearrange("(t p c) k -> t p (c k)", p=P, c=ci)
    IMv = IM.rearrange("(t p c) k -> t p (c k)", p=P, c=cf)
    TTv = TT.rearrange("(t p c) k -> t p (c k)", p=P, c=cf)
    F6v = F6.rearrange("(t p c) k -> t p (c k)", p=P, c=cf)

    mul, add, sub = (mybir.AluOpType.mult, mybir.AluOpType.add,
                     mybir.AluOpType.subtract)

    with tile.TileContext(nc) as tc:
        with tc.tile_pool(name="sbuf", bufs=2) as pool:
            for t in range(nti):
                xt = pool.tile([P, ci * 8], F32, tag="xt")
                nc.sync.dma_start(out=xt[:], in_=Xv[t])
                x3 = xt[:].rearrange("p (c k) -> p c k", k=8)
                sa, sb = x3[:, :, 3], x3[:, :, 7]
                dn = pool.tile([P, ci], F32, tag="dn")
                nc.vector.tensor_tensor(out=dn[:], in0=sa, in1=sb, op=sub)
                r = pool.tile([P, ci], F32, tag="r")
                nc.vector.reciprocal(out=r[:], in_=dn[:])
                w1 = pool.tile([P, ci], F32, tag="w1")
                nc.vector.tensor_tensor(out=w1[:], in0=sa, in1=r[:], op=mul)
                w0n = pool.tile([P, ci], F32, tag="w0n")
                nc.vector.tensor_tensor(out=w0n[:], in0=sb, in1=r[:], op=mul)
                ot = pool.tile([P, ci * 3], F32, tag="ot")
                o3 = ot[:].rearrange("p (c k) -> p c k", k=3)
                ta = pool.tile([P, ci], F32, tag="ta")
                tb = pool.tile([P, ci], F32, tag="tb")
                for k in range(3):
                    nc.vector.tensor_tensor(
                        out=ta[:], in0=x3[:, :, k], in1=w0n[:], op=mul)
                    nc.vector.tensor_tensor(
                        out=tb[:], in0=x3[:, :, 4 + k], in1=w1[:], op=mul)
                    nc.vector.tensor_tensor(
                        out=o3[:, :, k], in0=tb[:], in1=ta[:], op=sub)
                nc.sync.dma_start(out=VOv[t], in_=ot[:])

            for t in range(ntf):
                imt = pool.tile([P, cf * 6], F32, tag="imt")
                ttt = pool.tile([P, cf * 6], F32, tag="ttt")
                nc.sync.dma_start(out=imt[:], in_=IMv[t])
                nc.sync.dma_start(out=ttt[:], in_=TTv[t])
                im3 = imt[:].rearrange("p (c k) -> p c k", k=6)
                f6t = pool.tile([P, cf * 6], F32, tag="f6t")
                tmp = pool.tile([P, cf * 6], F32, tag="tmp")
                for j in range(6):
                    nc.vector.tensor_scalar(
                        out=tmp[:], in0=ttt[:], scalar1=float(j), scalar2=None,
                        op0=mybir.AluOpType.is_equal)
                    imj = im3[:, :, j].to_broadcast([P, cf, 6])
                    tmp3 = tmp[:].rearrange("p (c k) -> p c k", k=6)
                    if j == 0:
                        f63 = f6t[:].rearrange("p (c k) -> p c k", k=6)
                        nc.vector.tensor_tensor(
                            out=f63[:, :, :], in0=tmp3[:, :, :], in1=imj, op=mul)
                    else:
                        nc.vector.tensor_tensor(
                            out=tmp3[:, :, :], in0=tmp3[:, :, :], in1=imj, op=mul)
                        nc.vector.tensor_tensor(
                            out=f6t[:], in0=f6t[:], in1=tmp[:], op=add)
                nc.sync.dma_start(out=F6v[t], in_=f6t[:])
    nc.compile()
    return nc


def kernel(verts, sdf, deform, indices, grid_res):
    verts = np.ascontiguousarray(verts, dtype=np.float32)
    sdf = np.ascontiguousarray(sdf, dtype=np.float32)
    deform = np.ascontiguousarray(deform, dtype=np.float32)
    indices = np.ascontiguousarray(indices, dtype=np.int32)
    Nv = verts.shape[0]
    scale = 1.0 / float(grid_res)

    # ---------------- Launch A: pos_sdf table + occupancy ----------------
    P = 128
    rows_pc = -(-Nv // (NCORES * P)) * P          # per-core rows, 128-aligned
    tot = rows_pc * NCORES
    vp = np.zeros((tot, 3), np.float32); vp[:Nv] = verts
    dp = np.zeros((tot, 3), np.float32); dp[:Nv] = deform
    sp = np.full((tot,), -1.0, np.float32); sp[:Nv] = sdf
    ncA = _build_launch_a(rows_pc, scale)
    in_maps = [{"verts": vp[c * rows_pc:(c + 1) * rows_pc],
                "deform": dp[c * rows_pc:(c + 1) * rows_pc],
                "sdf": sp[c * rows_pc:(c + 1) * rows_pc]}
               for c in range(NCORES)]
    resA = _run(ncA, in_maps)
    pos_sdf = np.concatenate([resA[c]["pos_sdf"] for c in range(NCORES)])[:Nv]
    occ = np.concatenate([resA[c]["occ"] for c in range(NCORES)])[:Nv] > 0.5

    # ---------------- Host: codes, edges, dedup ----------------
    occ_f = occ[indices]                                    # [Nt,4]
    tetcode = (occ_f * np.array([1, 2, 4, 8], np.int32)).sum(-1).astype(np.int32)
    valid = (tetcode > 0) & (tetcode < 15)
    vt = indices[valid]
    codes_v = tetcode[valid]
    Fv = len(vt)

    a_full = vt[:, EI]; b_full = vt[:, EJ]
    lo = np.minimum(a_full, b_full).astype(np.int64)
    hi = np.maximum(a_full, b_full).astype(np.int64)
    keys_full = lo * Nv + hi
    crossing = CROSS[codes_v]
    keys_c = keys_full[crossing]

    if len(keys_c) == 0:
        return (np.zeros((0, 3), np.float32), np.zeros((0, 3), np.int32))

    order = np.argsort(keys_c)
    skeys = keys_c[order]
    flag = np.empty(len(skeys), bool); flag[0] = True
    np.not_equal(skeys[1:], skeys[:-1], out=flag[1:])
    group_sorted = np.cumsum(flag) - 1
    inv = np.empty(len(skeys), np.int64)
    inv[order] = group_sorted
    u = skeys[flag]
    M = len(u)
    ua = (u // Nv).astype(np.int64)
    ub = (u % Nv).astype(np.int64)

    idx_map = np.zeros((Fv, 6), np.float32)
    idx_map[crossing] = inv.astype(np.float32)
    ttf = TRIANGLE_TABLE[codes_v].astype(np.float32)
    ntri = NUM_TRI[codes_v]

    # ---------------- Launch B: interp + face select ----------------
    CI, CF = 512, 512
    qi, qf = P * CI, P * CF
    mq = max(1, -(-M // (NCORES * qi))) * qi
    fq = max(1, -(-Fv // (NCORES * qf))) * qf
    Mp, Fp = mq * NCORES, fq * NCORES

    X = np.zeros((Mp, 8), np.float32)
    X[:, 3] = 1.0; X[:, 7] = -1.0                 # pad rows: denom=2, no inf
    X[:M, 0:4] = pos_sdf[ua]
    X[:M, 4:8] = pos_sdf[ub]
    IMp = np.zeros((Fp, 6), np.float32); IMp[:Fv] = idx_map
    TTp = np.full((Fp, 6), -1.0, np.float32); TTp[:Fv] = ttf

    ncB = _build_launch_b(mq, fq, CI, CF)
    in_maps = [{"X": X[c * mq:(c + 1) * mq],
                "IM": IMp[c * fq:(c + 1) * fq],
                "TT": TTp[c * fq:(c + 1) * fq]}
               for c in range(NCORES)]
    resB = _run(ncB, in_maps)
    verts_out = np.concatenate([resB[c]["VO"] for c in range(NCORES)])[:M]
    faces6 = np.concatenate([resB[c]["F6"] for c in range(NCORES)])[:Fv]
    faces6 = faces6.astype(np.int32)

    f1 = faces6[ntri == 1][:, :3]
    f2 = faces6[ntri == 2].reshape(-1, 3)
    faces = np.concatenate([f1, f2], axis=0)
    return (verts_out, faces)


# revision 4
# speedup vs baseline: 1.0302x; 1.0302x over previous
"""DMTet geometry extraction on 8 Trainium2 NeuronCores.

Pipeline:
  Launch A (device, verts sharded): pos = verts + tanh(deform)/grid_res,
      packed pos_sdf table [Nv,4] + occupancy (sdf>0) per vertex.
  Host: tet occupancy codes -> valid (crossing) tets -> crossing edges ->
      canonical-sort + dedup (the global `unique` step) -> interp edge list,
      idx_map / triangle-table rows, and the random-access row gathers
      (no fast large-table gather primitive exists in this Bass stack, so
      index-space work stays on host).
  Launch B (device, edges + tets sharded): linear interpolation along
      crossing edges (reciprocal/weights/lerp) and the 6-way triangle-table
      select producing face vertex ids.
"""
import os
import numpy as np

import concourse.bacc as bacc
import concourse.bass as bass
import concourse.mybir as mybir
import concourse.tile as tile
from concourse import bass_utils

NCORES = 8
F32 = mybir.dt.float32

TRIANGLE_TABLE = np.array([
    [-1, -1, -1, -1, -1, -1], [1, 0, 2, -1, -1, -1], [4, 0, 3, -1, -1, -1],
    [1, 4, 2, 1, 3, 4], [3, 1, 5, -1, -1, -1], [2, 3, 0, 2, 5, 3],
    [1, 4, 0, 1, 5, 4], [4, 2, 5, -1, -1, -1], [4, 5, 2, -1, -1, -1],
    [4, 1, 0, 4, 5, 1], [3, 2, 0, 3, 5, 2], [1, 3, 5, -1, -1, -1],
    [4, 1, 2, 4, 3, 1], [3, 0, 4, -1, -1, -1], [2, 0, 1, -1, -1, -1],
    [-1, -1, -1, -1, -1, -1]], dtype=np.int32)
NUM_TRI = np.array([0, 1, 1, 2, 1, 2, 2, 1, 1, 2, 2, 1, 2, 1, 1, 0],
                   dtype=np.int32)
EI = np.array([0, 0, 0, 1, 1, 2], dtype=np.int32)
EJ = np.array([1, 2, 3, 2, 3, 3], dtype=np.int32)
_BITS = ((np.arange(16)[:, None] >> np.arange(4)[None, :]) & 1).astype(np.int32)
CROSS = _BITS[:, EI] != _BITS[:, EJ]  # [16, 6] bool

EXEC_NS = []  # per-launch max-over-cores HW exec time (filled when tracing)


def _run(nc, in_maps):
    trace = os.environ.get("DMTET_TRACE") == "1"
    kw = {}
    if trace:
        kw = dict(trace=True, trace_cores=list(range(NCORES)))
    res = bass_utils.run_bass_kernel_spmd(
        nc, in_maps, core_ids=list(range(NCORES)), **kw)
    if trace:
        EXEC_NS.append(res.exec_time_ns)
    return res.results


def _build_launch_a(rows_pc, scale):
    """Per core: pos_sdf [rows_pc, 4] f32 and occ [rows_pc] f32 from
    verts/deform [rows_pc, 3] + sdf [rows_pc]. rows_pc % 128 == 0."""
    P = 128
    C = rows_pc // P
    nc = bacc.Bacc("TRN2", target_bir_lowering=False, debug=False)
    v = nc.dram_tensor("verts", [rows_pc, 3], F32, kind="ExternalInput").ap()
    d = nc.dram_tensor("deform", [rows_pc, 3], F32, kind="ExternalInput").ap()
    s = nc.dram_tensor("sdf", [rows_pc], F32, kind="ExternalInput").ap()
    ps = nc.dram_tensor("pos_sdf", [rows_pc, 4], F32, kind="ExternalOutput").ap()
    oc = nc.dram_tensor("occ", [rows_pc], F32, kind="ExternalOutput").ap()

    vv = v.rearrange("(p c) k -> p (c k)", p=P)
    dv = d.rearrange("(p c) k -> p (c k)", p=P)
    sv = s.rearrange("(p c) -> p c", p=P)
    pv = ps.rearrange("(p c) k -> p (c k)", p=P)
    ov = oc.rearrange("(p c) -> p c", p=P)

    with tile.TileContext(nc) as tc:
        with tc.tile_pool(name="sbuf", bufs=1) as pool:
            vt = pool.tile([P, C * 3], F32)
            dt = pool.tile([P, C * 3], F32)
            st = pool.tile([P, C], F32)
            nc.sync.dma_start(out=vt[:], in_=vv)
            nc.sync.dma_start(out=dt[:], in_=dv)
            nc.sync.dma_start(out=st[:], in_=sv)
            th = pool.tile([P, C * 3], F32)
            nc.scalar.activation(th[:], dt[:], mybir.ActivationFunctionType.Tanh)
            nc.vector.tensor_scalar_mul(th[:], th[:], float(scale))
            out4 = pool.tile([P, C * 4], F32)
            o3 = out4[:].rearrange("p (c k) -> p c k", k=4)
            v3 = vt[:].rearrange("p (c k) -> p c k", k=3)
            t3 = th[:].rearrange("p (c k) -> p c k", k=3)
            for k in range(3):
                nc.vector.tensor_tensor(
                    out=o3[:, :, k], in0=v3[:, :, k], in1=t3[:, :, k],
                    op=mybir.AluOpType.add)
            nc.vector.tensor_copy(out=o3[:, :, 3], in_=st[:])
            ot = pool.tile([P, C], F32)
            nc.vector.tensor_scalar(
                out=ot[:], in0=st[:], scalar1=0.0, scalar2=None,
                op0=mybir.AluOpType.is_gt)
            nc.sync.dma_start(out=pv, in_=out4[:])
            nc.sync.dma_start(out=ov, in_=ot[:])
    nc.compile()
    return nc


def _build_launch_b(mq, f1q, f2q, ci, cf):
    """Per core: interp over X [mq, 8] -> VO [mq, 3]; face select over
    (IM1 [f1q,6], TT1 [f1q,3]) -> FO1 [f1q,3] and (IM2 [f2q,6], TT2 [f2q,6])
    -> FO2 [f2q,6]. All row counts are multiples of 128*ci / 128*cf."""
    P = 128
    nti = mq // (P * ci)
    nc = bacc.Bacc("TRN2", target_bir_lowering=False, debug=False)
    X = nc.dram_tensor("X", [mq, 8], F32, kind="ExternalInput").ap()
    IM1 = nc.dram_tensor("IM1", [f1q, 6], F32, kind="ExternalInput").ap()
    TT1 = nc.dram_tensor("TT1", [f1q, 3], F32, kind="ExternalInput").ap()
    IM2 = nc.dram_tensor("IM2", [f2q, 6], F32, kind="ExternalInput").ap()
    TT2 = nc.dram_tensor("TT2", [f2q, 6], F32, kind="ExternalInput").ap()
    VO = nc.dram_tensor("VO", [mq, 3], F32, kind="ExternalOutput").ap()
    FO1 = nc.dram_tensor("FO1", [f1q, 3], F32, kind="ExternalOutput").ap()
    FO2 = nc.dram_tensor("FO2", [f2q, 6], F32, kind="ExternalOutput").ap()

    Xv = X.rearrange("(t p c) k -> t p (c k)", p=P, c=ci)
    VOv = VO.rearrange("(t p c) k -> t p (c k)", p=P, c=ci)

    mul, add, sub = (mybir.AluOpType.mult, mybir.AluOpType.add,
                     mybir.AluOpType.subtract)
    iseq = mybir.AluOpType.is_equal

    def face_tiles(pool, imv, ttv, fov, ntiles, nslots):
        for t in range(ntiles):
            imt = pool.tile([P, cf * 6], F32, tag="imt")
            ttt = pool.tile([P, cf * nslots], F32, tag="ttt")
            nc.sync.dma_start(out=imt[:], in_=imv[t])
            nc.sync.dma_start(out=ttt[:], in_=ttv[t])
            im3 = imt[:].rearrange("p (c k) -> p c k", k=6)
            fot = pool.tile([P, cf * nslots], F32, tag="fot")
            tmp = pool.tile([P, cf * nslots], F32, tag="tmp")
            tt3 = ttt[:].rearrange("p (c k) -> p c k", k=nslots)
            tmp3 = tmp[:].rearrange("p (c k) -> p c k", k=nslots)
            fo3 = fot[:].rearrange("p (c k) -> p c k", k=nslots)
            for j in range(6):
                imj = im3[:, :, j].to_broadcast([P, cf, nslots])
                dst3 = fo3 if j == 0 else tmp3
                nc.vector.scalar_tensor_tensor(
                    out=dst3[:, :, :], in0=tt3[:, :, :], scalar=float(j),
                    in1=imj, op0=iseq, op1=mul)
                if j > 0:
                    nc.vector.tensor_tensor(
                        out=fot[:], in0=fot[:], in1=tmp[:], op=add)
            nc.sync.dma_start(out=fov[t], in_=fot[:])

    with tile.TileContext(nc) as tc:
        with tc.tile_pool(name="interp", bufs=2) as pool:
            for t in range(nti):
                xt = pool.tile([P, ci * 8], F32, tag="xt")
                nc.sync.dma_start(out=xt[:], in_=Xv[t])
                x3 = xt[:].rearrange("p (c k) -> p c k", k=8)
                sa, sb = x3[:, :, 3], x3[:, :, 7]
                dn = pool.tile([P, ci], F32, tag="dn")
                nc.vector.tensor_tensor(out=dn[:], in0=sa, in1=sb, op=sub)
                r = pool.tile([P, ci], F32, tag="r")
                nc.vector.reciprocal(out=r[:], in_=dn[:])
                w1 = pool.tile([P, ci], F32, tag="w1")
                nc.vector.tensor_tensor(out=w1[:], in0=sa, in1=r[:], op=mul)
                w0n = pool.tile([P, ci], F32, tag="w0n")
                nc.vector.tensor_tensor(out=w0n[:], in0=sb, in1=r[:], op=mul)
                ot = pool.tile([P, ci * 3], F32, tag="ot")
                o3 = ot[:].rearrange("p (c k) -> p c k", k=3)
                ta = pool.tile([P, ci], F32, tag="ta")
                for k in range(3):
                    nc.vector.tensor_tensor(
                        out=ta[:], in0=x3[:, :, k], in1=w0n[:], op=mul)
                    nc.vector.tensor_tensor(
                        out=o3[:, :, k], in0=x3[:, :, 4 + k], in1=w1[:], op=mul)
                    nc.vector.tensor_tensor(
                        out=o3[:, :, k], in0=o3[:, :, k], in1=ta[:], op=sub)
                nc.sync.dma_start(out=VOv[t], in_=ot[:])

        with tc.tile_pool(name="faces", bufs=2) as fpool:
            face_tiles(fpool,
                       IM1.rearrange("(t p c) k -> t p (c k)", p=P, c=cf),
                       TT1.rearrange("(t p c) k -> t p (c k)", p=P, c=cf),
                       FO1.rearrange("(t p c) k -> t p (c k)", p=P, c=cf),
                       f1q // (P * cf), 3)
            face_tiles(fpool,
                       IM2.rearrange("(t p c) k -> t p (c k)", p=P, c=cf),
                       TT2.rearrange("(t p c) k -> t p (c k)", p=P, c=cf),
                       FO2.rearrange("(t p c) k -> t p (c k)", p=P, c=cf),
                       f2q // (P * cf), 6)
    nc.compile()
    return nc


def kernel(verts, sdf, deform, indices, grid_res):
    verts = np.ascontiguousarray(verts, dtype=np.float32)
    sdf = np.ascontiguousarray(sdf, dtype=np.float32)
    deform = np.ascontiguousarray(deform, dtype=np.float32)
    indices = np.ascontiguousarray(indices, dtype=np.int32)
    Nv = verts.shape[0]
    scale = 1.0 / float(grid_res)

    # ---------------- Launch A: pos_sdf table + occupancy ----------------
    P = 128
    rows_pc = -(-Nv // (NCORES * P)) * P          # per-core rows, 128-aligned
    tot = rows_pc * NCORES
    vp = np.zeros((tot, 3), np.float32); vp[:Nv] = verts
    dp = np.zeros((tot, 3), np.float32); dp[:Nv] = deform
    sp = np.full((tot,), -1.0, np.float32); sp[:Nv] = sdf
    ncA = _build_launch_a(rows_pc, scale)
    in_maps = [{"verts": vp[c * rows_pc:(c + 1) * rows_pc],
                "deform": dp[c * rows_pc:(c + 1) * rows_pc],
                "sdf": sp[c * rows_pc:(c + 1) * rows_pc]}
               for c in range(NCORES)]
    resA = _run(ncA, in_maps)
    pos_sdf = np.concatenate([resA[c]["pos_sdf"] for c in range(NCORES)])[:Nv]
    occ = np.concatenate([resA[c]["occ"] for c in range(NCORES)])[:Nv] > 0.5

    # ---------------- Host: codes, edges, dedup ----------------
    occ_f = occ[indices]                                    # [Nt,4]
    tetcode = (occ_f * np.array([1, 2, 4, 8], np.int32)).sum(-1).astype(np.int32)
    valid = (tetcode > 0) & (tetcode < 15)
    vt = indices[valid]
    codes_v = tetcode[valid]
    Fv = len(vt)

    a_full = vt[:, EI]; b_full = vt[:, EJ]
    lo = np.minimum(a_full, b_full).astype(np.int64)
    hi = np.maximum(a_full, b_full).astype(np.int64)
    keys_full = lo * Nv + hi
    crossing = CROSS[codes_v]
    keys_c = keys_full[crossing]

    if len(keys_c) == 0:
        return (np.zeros((0, 3), np.float32), np.zeros((0, 3), np.int32))

    order = np.argsort(keys_c)
    skeys = keys_c[order]
    flag = np.empty(len(skeys), bool); flag[0] = True
    np.not_equal(skeys[1:], skeys[:-1], out=flag[1:])
    group_sorted = np.cumsum(flag) - 1
    inv = np.empty(len(skeys), np.int64)
    inv[order] = group_sorted
    u = skeys[flag]
    M = len(u)
    ua = (u // Nv).astype(np.int64)
    ub = (u % Nv).astype(np.int64)

    idx_map = np.zeros((Fv, 6), np.float32)
    idx_map[crossing] = inv.astype(np.float32)
    ttf = TRIANGLE_TABLE[codes_v].astype(np.float32)
    ntri = NUM_TRI[codes_v]
    m1 = ntri == 1
    m2 = ntri == 2
    im1 = idx_map[m1]; tt1 = ttf[m1][:, :3]
    im2 = idx_map[m2]; tt2 = ttf[m2]
    n1, n2 = len(im1), len(im2)

    # ---------------- Launch B: interp + face select ----------------
    CI, CF = 512, 512
    qi, qf = P * CI, P * CF
    mq = max(1, -(-M // (NCORES * qi))) * qi
    f1q = max(1, -(-n1 // (NCORES * qf))) * qf
    f2q = max(1, -(-n2 // (NCORES * qf))) * qf
    Mp, F1p, F2p = mq * NCORES, f1q * NCORES, f2q * NCORES

    X = np.zeros((Mp, 8), np.float32)
    X[:, 3] = 1.0; X[:, 7] = -1.0                 # pad rows: denom=2, no inf
    X[:M, 0:4] = pos_sdf[ua]
    X[:M, 4:8] = pos_sdf[ub]
    IM1p = np.zeros((F1p, 6), np.float32); IM1p[:n1] = im1
    TT1p = np.full((F1p, 3), -1.0, np.float32); TT1p[:n1] = tt1
    IM2p = np.zeros((F2p, 6), np.float32); IM2p[:n2] = im2
    TT2p = np.full((F2p, 6), -1.0, np.float32); TT2p[:n2] = tt2

    ncB = _build_launch_b(mq, f1q, f2q, CI, CF)
    in_maps = [{"X": X[c * mq:(c + 1) * mq],
                "IM1": IM1p[c * f1q:(c + 1) * f1q],
                "TT1": TT1p[c * f1q:(c + 1) * f1q],
                "IM2": IM2p[c * f2q:(c + 1) * f2q],
                "TT2": TT2p[c * f2q:(c + 1) * f2q]}
               for c in range(NCORES)]
    resB = _run(ncB, in_maps)
    verts_out = np.concatenate([resB[c]["VO"] for c in range(NCORES)])[:M]
    f1 = np.concatenate([resB[c]["FO1"] for c in range(NCORES)])[:n1]
    f2 = np.concatenate([resB[c]["FO2"] for c in range(NCORES)])[:n2]
    faces = np.concatenate(
        [f1.astype(np.int32), f2.astype(np.int32).reshape(-1, 3)], axis=0)
    return (verts_out, faces)


# revision 6
# speedup vs baseline: 1.4111x; 1.3697x over previous
"""DMTet geometry extraction on 8 Trainium2 NeuronCores.

Pipeline:
  Launch A (device, verts sharded): pos = verts + tanh(deform)/grid_res
      (planar), occupancy (sdf>0) per vertex.
  Host: tet occupancy codes -> valid (crossing) tets -> crossing edges ->
      canonical-sort + dedup (the global `unique` step) -> interp edge list,
      per-tet crossing-edge id groups, rank-remapped triangle-table rows,
      and the random-access row gathers (no fast large-table gather
      primitive exists in this Bass stack, so index-space work is host-side).
  Launch B (device, edges + tets sharded): linear interpolation along
      crossing edges (reciprocal/weights/lerp) and the rank-select
      producing face vertex ids (3-way for 1-tri tets, 4-way for 2-tri).
"""
import os
import numpy as np

import concourse.bacc as bacc
import concourse.bass as bass
import concourse.mybir as mybir
import concourse.tile as tile
from concourse import bass_utils

NCORES = 8
F32 = mybir.dt.float32

TRIANGLE_TABLE = np.array([
    [-1, -1, -1, -1, -1, -1], [1, 0, 2, -1, -1, -1], [4, 0, 3, -1, -1, -1],
    [1, 4, 2, 1, 3, 4], [3, 1, 5, -1, -1, -1], [2, 3, 0, 2, 5, 3],
    [1, 4, 0, 1, 5, 4], [4, 2, 5, -1, -1, -1], [4, 5, 2, -1, -1, -1],
    [4, 1, 0, 4, 5, 1], [3, 2, 0, 3, 5, 2], [1, 3, 5, -1, -1, -1],
    [4, 1, 2, 4, 3, 1], [3, 0, 4, -1, -1, -1], [2, 0, 1, -1, -1, -1],
    [-1, -1, -1, -1, -1, -1]], dtype=np.int32)
NUM_TRI = np.array([0, 1, 1, 2, 1, 2, 2, 1, 1, 2, 2, 1, 2, 1, 1, 0],
                   dtype=np.int32)
EI = np.array([0, 0, 0, 1, 1, 2], dtype=np.int32)
EJ = np.array([1, 2, 3, 2, 3, 3], dtype=np.int32)
_BITS = ((np.arange(16)[:, None] >> np.arange(4)[None, :]) & 1).astype(np.int32)
CROSS = _BITS[:, EI] != _BITS[:, EJ]          # [16, 6] bool
# rank of slot s among the (sorted) crossing slots of config c; -1 elsewhere
RANK = np.full((16, 6), -1, np.int32)
for _c in range(16):
    for _r, _s in enumerate(np.nonzero(CROSS[_c])[0]):
        RANK[_c, _s] = _r
# triangle table remapped to crossing-edge ranks (per config)
TT_RANK = np.where(TRIANGLE_TABLE >= 0,
                   np.take_along_axis(
                       RANK, np.maximum(TRIANGLE_TABLE, 0), axis=1),
                   -1).astype(np.int32)

EXEC_NS = []  # per-launch max-over-cores HW exec time (filled when tracing)


def _run(nc, in_maps):
    trace = os.environ.get("DMTET_TRACE") == "1"
    kw = {}
    if trace:
        kw = dict(trace=True, trace_cores=list(range(NCORES)))
    res = bass_utils.run_bass_kernel_spmd(
        nc, in_maps, core_ids=list(range(NCORES)), **kw)
    if trace:
        EXEC_NS.append(res.exec_time_ns)
    return res.results


def _build_launch_a(rows_pc, scale):
    """Per core (rows_pc % 128 == 0): planar pos x/y/z + occ from planar
    verts/deform components + sdf."""
    P = 128
    C = rows_pc // P
    nc = bacc.Bacc("TRN2", target_bir_lowering=False, debug=False)
    vin = [nc.dram_tensor(f"v{k}", [rows_pc], F32, kind="ExternalInput").ap()
           for k in "xyz"]
    din = [nc.dram_tensor(f"d{k}", [rows_pc], F32, kind="ExternalInput").ap()
           for k in "xyz"]
    s = nc.dram_tensor("sdf", [rows_pc], F32, kind="ExternalInput").ap()
    pout = [nc.dram_tensor(f"p{k}", [rows_pc], F32, kind="ExternalOutput").ap()
            for k in "xyz"]
    oc = nc.dram_tensor("occ", [rows_pc], F32, kind="ExternalOutput").ap()

    with tile.TileContext(nc) as tc:
        with tc.tile_pool(name="sbuf", bufs=1) as pool:
            vt = pool.tile([P, 3 * C], F32)
            dt = pool.tile([P, 3 * C], F32)
            st = pool.tile([P, C], F32)
            for k in range(3):
                nc.sync.dma_start(out=vt[:, k * C:(k + 1) * C],
                                  in_=vin[k].rearrange("(p c) -> p c", p=P))
                nc.sync.dma_start(out=dt[:, k * C:(k + 1) * C],
                                  in_=din[k].rearrange("(p c) -> p c", p=P))
            nc.sync.dma_start(out=st[:], in_=s.rearrange("(p c) -> p c", p=P))
            th = pool.tile([P, 3 * C], F32)
            nc.scalar.activation(th[:], dt[:], mybir.ActivationFunctionType.Tanh)
            nc.vector.tensor_scalar_mul(th[:], th[:], float(scale))
            po = pool.tile([P, 3 * C], F32)
            nc.vector.tensor_tensor(out=po[:], in0=vt[:], in1=th[:],
                                    op=mybir.AluOpType.add)
            ot = pool.tile([P, C], F32)
            nc.vector.tensor_scalar(
                out=ot[:], in0=st[:], scalar1=0.0, scalar2=None,
                op0=mybir.AluOpType.is_gt)
            for k in range(3):
                nc.sync.dma_start(out=pout[k].rearrange("(p c) -> p c", p=P),
                                  in_=po[:, k * C:(k + 1) * C])
            nc.sync.dma_start(out=oc.rearrange("(p c) -> p c", p=P), in_=ot[:])
    nc.compile()
    return nc


def _build_launch_b(mq, f1q, f2q, ci, cf):
    """Per core: interp over 8 planar streams [mq] -> planar vx/vy/vz [mq];
    rank-select faces: (IM3 [f1q,3], TT1 [f1q,3]) -> FO1 [f1q,3] and
    (IM4 [f2q,4], TT2 [f2q,6]) -> FO2 [f2q,6]."""
    P = 128
    nti = mq // (P * ci)
    nc = bacc.Bacc("TRN2", target_bir_lowering=False, debug=False)
    xin = [nc.dram_tensor(n, [mq], F32, kind="ExternalInput").ap()
           for n in ("pax", "pay", "paz", "saa", "pbx", "pby", "pbz", "sbb")]
    IM3 = nc.dram_tensor("IM3", [f1q, 3], F32, kind="ExternalInput").ap()
    TT1 = nc.dram_tensor("TT1", [f1q, 3], F32, kind="ExternalInput").ap()
    IM4 = nc.dram_tensor("IM4", [f2q, 4], F32, kind="ExternalInput").ap()
    TT2 = nc.dram_tensor("TT2", [f2q, 6], F32, kind="ExternalInput").ap()
    vout = [nc.dram_tensor(n, [mq], F32, kind="ExternalOutput").ap()
            for n in ("vx", "vy", "vz")]
    FO1 = nc.dram_tensor("FO1", [f1q, 3], F32, kind="ExternalOutput").ap()
    FO2 = nc.dram_tensor("FO2", [f2q, 6], F32, kind="ExternalOutput").ap()

    xv = [x.rearrange("(t p c) -> t p c", p=P, c=ci) for x in xin]
    vv = [x.rearrange("(t p c) -> t p c", p=P, c=ci) for x in vout]

    mul, add, sub = (mybir.AluOpType.mult, mybir.AluOpType.add,
                     mybir.AluOpType.subtract)
    iseq = mybir.AluOpType.is_equal

    def face_tiles(pool, imv, ttv, fov, ntiles, nout, nj):
        for t in range(ntiles):
            imt = pool.tile([P, cf * nj], F32, tag="imt")
            ttt = pool.tile([P, cf * nout], F32, tag="ttt")
            nc.sync.dma_start(out=imt[:], in_=imv[t])
            nc.sync.dma_start(out=ttt[:], in_=ttv[t])
            im3 = imt[:].rearrange("p (c k) -> p c k", k=nj)
            fot = pool.tile([P, cf * nout], F32, tag="fot")
            tmp = pool.tile([P, cf * nout], F32, tag="tmp")
            tt3 = ttt[:].rearrange("p (c k) -> p c k", k=nout)
            tmp3 = tmp[:].rearrange("p (c k) -> p c k", k=nout)
            fo3 = fot[:].rearrange("p (c k) -> p c k", k=nout)
            for j in range(nj):
                imj = im3[:, :, j].to_broadcast([P, cf, nout])
                dst3 = fo3 if j == 0 else tmp3
                nc.vector.scalar_tensor_tensor(
                    out=dst3[:, :, :], in0=tt3[:, :, :], scalar=float(j),
                    in1=imj, op0=iseq, op1=mul)
                if j > 0:
                    nc.vector.tensor_tensor(
                        out=fot[:], in0=fot[:], in1=tmp[:], op=add)
            nc.sync.dma_start(out=fov[t], in_=fot[:])

    with tile.TileContext(nc) as tc:
        with tc.tile_pool(name="interp", bufs=2) as pool:
            for t in range(nti):
                xt = [pool.tile([P, ci], F32, tag=f"x{i}", name=f"x{i}")
                      for i in range(8)]
                for i in range(8):
                    nc.sync.dma_start(out=xt[i][:], in_=xv[i][t])
                sa, sb = xt[3][:], xt[7][:]
                dn = pool.tile([P, ci], F32, tag="dn")
                nc.vector.tensor_tensor(out=dn[:], in0=sa, in1=sb, op=sub)
                r = pool.tile([P, ci], F32, tag="r")
                scr = pool.tile([P, ci], F32, tag="scr")
                nc.vector.reciprocal_approx_accurate(
                    out=r[:], in_=dn[:], scratch=scr[:])
                w1 = pool.tile([P, ci], F32, tag="w1")
                nc.vector.tensor_tensor(out=w1[:], in0=sa, in1=r[:], op=mul)
                w0n = pool.tile([P, ci], F32, tag="w0n")
                nc.vector.tensor_tensor(out=w0n[:], in0=sb, in1=r[:], op=mul)
                ta = pool.tile([P, ci], F32, tag="ta")
                for k in range(3):
                    ok = pool.tile([P, ci], F32, tag=f"o{k}")
                    nc.vector.tensor_tensor(
                        out=ta[:], in0=xt[k][:], in1=w0n[:], op=mul)
                    nc.vector.tensor_tensor(
                        out=ok[:], in0=xt[4 + k][:], in1=w1[:], op=mul)
                    nc.vector.tensor_tensor(
                        out=ok[:], in0=ok[:], in1=ta[:], op=sub)
                    nc.sync.dma_start(out=vv[k][t], in_=ok[:])

        with tc.tile_pool(name="faces", bufs=2) as fpool:
            face_tiles(fpool,
                       IM3.rearrange("(t p c) k -> t p (c k)", p=P, c=cf),
                       TT1.rearrange("(t p c) k -> t p (c k)", p=P, c=cf),
                       FO1.rearrange("(t p c) k -> t p (c k)", p=P, c=cf),
                       f1q // (P * cf), 3, 3)
            face_tiles(fpool,
                       IM4.rearrange("(t p c) k -> t p (c k)", p=P, c=cf),
                       TT2.rearrange("(t p c) k -> t p (c k)", p=P, c=cf),
                       FO2.rearrange("(t p c) k -> t p (c k)", p=P, c=cf),
                       f2q // (P * cf), 6, 4)
    nc.compile()
    return nc


def kernel(verts, sdf, deform, indices, grid_res):
    verts = np.ascontiguousarray(verts, dtype=np.float32)
    sdf = np.ascontiguousarray(sdf, dtype=np.float32)
    deform = np.ascontiguousarray(deform, dtype=np.float32)
    indices = np.ascontiguousarray(indices, dtype=np.int32)
    Nv = verts.shape[0]
    scale = 1.0 / float(grid_res)

    # ---------------- Launch A: pos (planar) + occupancy ----------------
    P = 128
    rows_pc = -(-Nv // (NCORES * P)) * P          # per-core rows, 128-aligned
    tot = rows_pc * NCORES
    planes = {}
    for k, name in enumerate("xyz"):
        a = np.zeros(tot, np.float32); a[:Nv] = verts[:, k]
        planes[f"v{name}"] = a
        a = np.zeros(tot, np.float32); a[:Nv] = deform[:, k]
        planes[f"d{name}"] = a
    sp = np.full(tot, -1.0, np.float32); sp[:Nv] = sdf
    planes["sdf"] = sp
    ncA = _build_launch_a(rows_pc, scale)
    in_maps = [{k: v[c * rows_pc:(c + 1) * rows_pc] for k, v in planes.items()}
               for c in range(NCORES)]
    resA = _run(ncA, in_maps)
    pos = [np.concatenate([resA[c][f"p{k}"] for c in range(NCORES)])[:Nv]
           for k in "xyz"]
    occ = np.concatenate([resA[c]["occ"] for c in range(NCORES)])[:Nv] > 0.5

    # ---------------- Host: codes, edges, dedup ----------------
    occ_f = occ[indices]                                    # [Nt,4]
    tetcode = (occ_f * np.array([1, 2, 4, 8], np.int32)).sum(-1).astype(np.int32)
    valid = (tetcode > 0) & (tetcode < 15)
    vt = indices[valid]
    codes_v = tetcode[valid]
    Fv = len(vt)

    a_full = vt[:, EI]; b_full = vt[:, EJ]
    lo = np.minimum(a_full, b_full).astype(np.int64)
    hi = np.maximum(a_full, b_full).astype(np.int64)
    keys_full = lo * Nv + hi
    crossing = CROSS[codes_v]
    keys_c = keys_full[crossing]

    if len(keys_c) == 0:
        return (np.zeros((0, 3), np.float32), np.zeros((0, 3), np.int32))

    order = np.argsort(keys_c)
    skeys = keys_c[order]
    flag = np.empty(len(skeys), bool); flag[0] = True
    np.not_equal(skeys[1:], skeys[:-1], out=flag[1:])
    group_sorted = np.cumsum(flag) - 1
    inv = np.empty(len(skeys), np.int64)
    inv[order] = group_sorted
    u = skeys[flag]
    M = len(u)
    ua = (u // Nv).astype(np.int64)
    ub = (u % Nv).astype(np.int64)

    invf = inv.astype(np.float32)
    counts = np.where(NUM_TRI[codes_v] == 2, 4, 3).astype(np.int64)
    starts = np.concatenate([[0], np.cumsum(counts)[:-1]])
    ntri = NUM_TRI[codes_v]
    m1 = ntri == 1
    m2 = ntri == 2
    im3 = invf[starts[m1][:, None] + np.arange(3)]          # [n1, 3]
    im4 = invf[starts[m2][:, None] + np.arange(4)]          # [n2, 4]
    ttr = TT_RANK[codes_v]
    tt1 = ttr[m1][:, :3].astype(np.float32)                 # [n1, 3]
    tt2 = ttr[m2].astype(np.float32)                        # [n2, 6]
    n1, n2 = len(im3), len(im4)

    # ---------------- Launch B: interp + face rank-select ----------------
    CI, CF = 512, 512
    qi, qf = P * CI, P * CF
    mq = max(1, -(-M // (NCORES * qi))) * qi
    f1q = max(1, -(-n1 // (NCORES * qf))) * qf
    f2q = max(1, -(-n2 // (NCORES * qf))) * qf
    Mp, F1p, F2p = mq * NCORES, f1q * NCORES, f2q * NCORES

    xs = {}
    for k, name in enumerate(("pax", "pay", "paz")):
        a = np.zeros(Mp, np.float32); a[:M] = pos[k][ua]
        xs[name] = a
    for k, name in enumerate(("pbx", "pby", "pbz")):
        a = np.zeros(Mp, np.float32); a[:M] = pos[k][ub]
        xs[name] = a
    a = np.ones(Mp, np.float32); a[:M] = sdf[ua]; xs["saa"] = a
    a = np.full(Mp, -1.0, np.float32); a[:M] = sdf[ub]; xs["sbb"] = a

    IM3p = np.zeros((F1p, 3), np.float32); IM3p[:n1] = im3
    TT1p = np.full((F1p, 3), -1.0, np.float32); TT1p[:n1] = tt1
    IM4p = np.zeros((F2p, 4), np.float32); IM4p[:n2] = im4
    TT2p = np.full((F2p, 6), -1.0, np.float32); TT2p[:n2] = tt2

    ncB = _build_launch_b(mq, f1q, f2q, CI, CF)
    in_maps = []
    for c in range(NCORES):
        m = {k: v[c * mq:(c + 1) * mq] for k, v in xs.items()}
        m["IM3"] = IM3p[c * f1q:(c + 1) * f1q]
        m["TT1"] = TT1p[c * f1q:(c + 1) * f1q]
        m["IM4"] = IM4p[c * f2q:(c + 1) * f2q]
        m["TT2"] = TT2p[c * f2q:(c + 1) * f2q]
        in_maps.append(m)
    resB = _run(ncB, in_maps)
    verts_out = np.empty((M, 3), np.float32)
    for k, name in enumerate(("vx", "vy", "vz")):
        verts_out[:, k] = np.concatenate(
            [resB[c][name] for c in range(NCORES)])[:M]
    f1 = np.concatenate([resB[c]["FO1"] for c in range(NCORES)])[:n1]
    f2 = np.concatenate([resB[c]["FO2"] for c in range(NCORES)])[:n2]
    faces = np.concatenate(
        [f1.astype(np.int32), f2.astype(np.int32).reshape(-1, 3)], axis=0)
    return (verts_out, faces)
